# revision 17
# baseline (speedup 1.0000x reference)
"""ECE loss kernel for Trainium2, data-parallel over 8 NeuronCores.

Host side shards + permutes samples (binning is permutation invariant) into
128-sample single-label "slots", swaps each slot's label class into column 0
(max/sum are class-permutation invariant, so accuracy becomes one column
compare), and casts logits to bf16 (ECE rel-err from bf16 ~3e-3, tol 2e-2).

Device per tile (128 partitions x 128 slots x 104 padded classes, bf16):
  ScalarE: one big exp(logits/T) instruction (13312 elems/partition).
  DVE:     pairwise tensor_tensor max/add trees (bf16 2x mode) down to
           per-sample max-exp and softmax denominators, fast reciprocal,
           conf/acc, and tile-pair-batched tensor_scalar threshold compares.
  PE:      four matmuls per tile accumulate per-(slot, bin) sums of
           (conf, acc) into four PSUM banks across all tiles.
Each core DMAs its [4,128,480] histogram out; the host extracts the diagonal
slot blocks, reduces 8 cores' 15-bin stats, and finishes ECE in float64.
"""

import hashlib
import sys

import numpy as np

sys.path.insert(0, "/opt/trn_rl_repo")

import ml_dtypes  # noqa: E402

from concourse import bacc, bass, mybir, tile  # noqa: E402
from concourse import bass_utils  # noqa: E402

P = 128          # partitions
G = 128          # slots (groups) per tile
TILE = P * G     # samples per tile (16384)
C = 100          # classes
CP = 104         # padded class stride (4B-aligned bf16 rows, tree-friendly)
NBINS = 15
N_CORES = 8
BIG = 80.0       # pad logit; exp(-80) ~ 1.8e-35 is harmless in f32/bf16
N_TOTAL = 2_000_000

F32 = mybir.dt.float32
BF16 = mybir.dt.bfloat16
BFNP = ml_dtypes.bfloat16
AX = mybir.AxisListType
ALU = mybir.AluOpType
ACTF = mybir.ActivationFunctionType


# ---------------------------------------------------------------- host layout

def build_plan(labels: np.ndarray, n_cores: int = N_CORES):
    """Deal samples round-robin per label so every core has the same number
    of 128-sample slots per label.  Returns (slot_labels, per-core sample
    index arrays with -1 for pad rows, tile count)."""
    labels = np.asarray(labels).astype(np.int64).ravel()
    order = np.argsort(labels, kind="stable")
    sorted_labels = labels[order]
    starts = np.searchsorted(sorted_labels, np.arange(C))
    ends = np.searchsorted(sorted_labels, np.arange(C), side="right")

    slot_labels = []
    core_chunks = [[] for _ in range(n_cores)]
    for k in range(C):
        idx_k = order[starts[k]:ends[k]]
        per_core = [idx_k[c::n_cores] for c in range(n_cores)]
        max_cnt = max(len(x) for x in per_core)
        slots_k = -(-max_cnt // P) if max_cnt > 0 else 0
        if slots_k == 0:
            continue
        padded = slots_k * P
        for c in range(n_cores):
            buf = np.full(padded, -1, dtype=np.int64)
            buf[: len(per_core[c])] = per_core[c]
            core_chunks[c].append(buf)
        slot_labels.extend([k] * slots_k)

    n_slots = len(slot_labels)
    pad_slots = (-n_slots) % (2 * G)
    if pad_slots:
        for c in range(n_cores):
            core_chunks[c].append(np.full(pad_slots * P, -1, dtype=np.int64))
        slot_labels.extend([0] * pad_slots)
        n_slots += pad_slots

    slot_labels = np.asarray(slot_labels, dtype=np.int64)
    core_idx = [np.concatenate(ch) for ch in core_chunks]
    T = n_slots // G
    return slot_labels, core_idx, T


def build_core_slab(logits: np.ndarray, idx: np.ndarray,
                    slot_labels: np.ndarray) -> np.ndarray:
    """One core's [T*TILE, CP] bf16 slab in device DMA order, with each
    slot's label class swapped into column 0."""
    S = len(slot_labels)
    arr = np.full((S * P, CP), -BIG, dtype=BFNP)
    arr[:, :C] = logits[np.maximum(idx, 0)].astype(BFNP)
    # swap label class into column 0 (rows of slot s all have the same label)
    ks = np.repeat(slot_labels, P)
    rows = np.arange(S * P)
    col0 = arr[rows, 0].copy()
    arr[rows, 0] = arr[rows, ks]
    arr[rows, ks] = col0
    pad_pos = np.nonzero(idx < 0)[0]
    if len(pad_pos):
        arr[pad_pos, :C] = BFNP(-BIG)
        arr[pad_pos, 0] = BFNP(BIG)
    # [S, P, CP] slot-major -> [T, P, G, CP] DMA order
    arr = arr.reshape(S // G, G, P, CP).transpose(0, 2, 1, 3)
    return np.ascontiguousarray(arr).reshape(-1, CP)


# ------------------------------------------------------------- device program

def build_program(T: int, n_cores: int = N_CORES):
    nc = bacc.Bacc("TRN2", target_bir_lowering=False, debug=False,
                   num_devices=n_cores)

    logits_d = nc.dram_tensor("logits", [T * TILE, CP], BF16,
                              kind="ExternalInput")
    tempr_d = nc.dram_tensor("tempr", [P, 1], F32, kind="ExternalInput")
    ident_d = nc.dram_tensor("ident", [P, P], BF16, kind="ExternalInput")
    gmat_d = nc.dram_tensor("gmat", [P, 14 * P], BF16, kind="ExternalInput")
    out_d = nc.dram_tensor("out", [4, P, 32 * NBINS], F32,
                           kind="ExternalOutput")

    thr_imm = [float(np.float32(BFNP(b / NBINS))) for b in range(NBINS)]
    assert T % 2 == 0

    with tile.TileContext(nc) as tc:
        with (
            tc.tile_pool(name="const", bufs=1) as const,
            tc.tile_pool(name="rawp", bufs=2) as rawp,
            tc.tile_pool(name="sb", bufs=2) as sbp,
            tc.tile_pool(name="ps", bufs=1, space="PSUM") as psp,
        ):
            tempr_t = const.tile([P, 1], F32)
            nc.sync.dma_start(tempr_t, tempr_d.ap())
            invT = const.tile([P, 1], F32)
            nc.vector.reciprocal(invT, tempr_t)
            warm = const.tile([P, 1], F32)
            nc.scalar.activation(warm, tempr_t, ACTF.Exp)
            ident_t = const.tile([P, P], BF16)
            nc.sync.dma_start(ident_t, ident_d.ap())
            gmat_t = const.tile([P, 14 * P], BF16)
            nc.sync.dma_start(gmat_t, gmat_d.ap())

            hists = [psp.tile([P, 32 * NBINS], F32, name=f"hist{q}")
                     for q in range(4)]

            logits_ap = logits_d.ap()

            Eh, t1h, u1h = {}, {}, {}

            def load_tile(t):
                """Emit dma+exp for tile t (sub-chunked fill; tile 0 also
                interleaves the L1 trees so DVE starts early)."""
                raw = rawp.tile([P, G * CP], BF16, tag="raw", name="raw")
                E = sbp.tile([P, G * CP], BF16, tag="E", name="E")
                Eh[t] = E
                E3 = E.rearrange("p (g c) -> p g c", g=G)
                src = logits_ap[t * TILE:(t + 1) * TILE,
                                :].rearrange("(p s) c -> p (s c)", p=P)
                nsub = {0: 8, 1: 4, 2: 2}.get(t, 1)
                gs = G // nsub
                if t == 0:
                    t1 = sbp.tile([P, G * 52], BF16, tag="t1", name="t1",
                                  bufs=1)
                    u1 = sbp.tile([P, G * 52], BF16, tag="u1", name="u1",
                                  bufs=1)
                    t1h[t], u1h[t] = t1, u1
                    t13 = t1.rearrange("p (g c) -> p g c", g=G)
                    u13 = u1.rearrange("p (g c) -> p g c", g=G)
                for c in range(nsub):
                    fsl = slice(c * gs * CP, (c + 1) * gs * CP)
                    gsl = slice(c * gs, (c + 1) * gs)
                    nc.sync.dma_start(raw[:, fsl], src[:, fsl])
                    nc.scalar.activation(E[:, fsl], raw[:, fsl], ACTF.Exp,
                                         scale=invT)
                    if t == 0:
                        nc.vector.tensor_tensor(u13[:, gsl, :],
                                                E3[:, gsl, 0:52],
                                                E3[:, gsl, 52:104], op=ALU.add)
                        nc.vector.tensor_tensor(t13[:, gsl, :],
                                                E3[:, gsl, 0:52],
                                                E3[:, gsl, 52:104], op=ALU.max)

            load_tile(0)
            for tp in range(T // 2):
                emax2 = sbp.tile([P, 2 * G], BF16, tag="emax2", name="emax2",
                                 bufs=1)
                conf2 = sbp.tile([P, 2 * G], BF16, tag="conf2", name="conf2",
                                 bufs=1)
                ST_h, pack_h, Rbt_h = [], [], []
                for h in range(2):
                    t = 2 * tp + h
                    if t + 1 < T:
                        load_tile(t + 1)  # ACT lookahead: exp before copies
                    E = Eh.pop(t)
                    E3 = E.rearrange("p (g c) -> p g c", g=G)

                    # ---- sum tree first (shortens the PE/ACT chain latency)
                    if t in u1h:
                        u1 = u1h.pop(t)
                        t1 = t1h.pop(t)
                        fresh_l1 = False
                    else:
                        t1 = sbp.tile([P, G * 52], BF16, tag="t1", name="t1",
                                      bufs=1)
                        u1 = sbp.tile([P, G * 52], BF16, tag="u1", name="u1",
                                      bufs=1)
                        fresh_l1 = True
                    u13 = u1.rearrange("p (g c) -> p g c", g=G)
                    t13 = t1.rearrange("p (g c) -> p g c", g=G)
                    if fresh_l1:
                        nc.vector.tensor_tensor(u13, E3[:, :, 0:52],
                                                E3[:, :, 52:104], op=ALU.add)
                    u2 = sbp.tile([P, G * 26], BF16, tag="u2", name="u2",
                                  bufs=1)
                    u23 = u2.rearrange("p (g c) -> p g c", g=G)
                    nc.vector.tensor_tensor(u23, u13[:, :, 0:26],
                                            u13[:, :, 26:52], op=ALU.add)
                    u3 = sbp.tile([P, G * 14], BF16, tag="u3", name="u3",
                                  bufs=1)
                    u33 = u3.rearrange("p (g c) -> p g c", g=G)
                    nc.vector.tensor_tensor(u33[:, :, 0:12], u23[:, :, 0:12],
                                            u23[:, :, 14:26], op=ALU.add)
                    nc.vector.tensor_copy(u33[:, :, 12:14], u23[:, :, 12:14])

                    # ---- PE transposes u3 -> PSUM; ScalarE stages to SBUF;
                    #      membership matmuls accumulate S^T[g, p] in PSUM
                    u3T = sbp.tile([P, 14 * P], BF16, tag="u3T", name="u3T")
                    for sg in range(4):
                        nblk = min(4, 14 - 4 * sg)
                        stg = psp.tile([P, 512], BF16, tag="stg", name="stg",
                                       bufs=2)
                        for bb in range(nblk):
                            j = 4 * sg + bb
                            nc.tensor.transpose(
                                stg[:, 128 * bb:128 * (bb + 1)],
                                u3[:, 128 * j:128 * (j + 1)], ident_t)
                        nc.scalar.copy(u3T[:, 512 * sg:512 * sg + 128 * nblk],
                                       stg[:, 0:128 * nblk])
                    STp = psp.tile([P, P], F32, tag="ST", name="STp", bufs=2)
                    for j in range(14):
                        nc.tensor.matmul(STp,
                                         lhsT=gmat_t[:, 128 * j:128 * (j + 1)],
                                         rhs=u3T[:, 128 * j:128 * (j + 1)],
                                         start=(j == 0), stop=(j == 13))
                    ST_h.append(STp)

                    # ---- max tree L2..L7 (overlapped splits keep alignment)
                    if fresh_l1:
                        nc.vector.tensor_tensor(t13, E3[:, :, 0:52],
                                                E3[:, :, 52:104], op=ALU.max)
                    t2 = sbp.tile([P, G * 26], BF16, tag="t2", name="t2",
                                  bufs=1)
                    t23 = t2.rearrange("p (g c) -> p g c", g=G)
                    nc.vector.tensor_tensor(t23, t13[:, :, 0:26],
                                            t13[:, :, 26:52], op=ALU.max)
                    t3 = sbp.tile([P, G * 14], BF16, tag="t3", name="t3",
                                  bufs=1)
                    t33 = t3.rearrange("p (g c) -> p g c", g=G)
                    nc.vector.tensor_tensor(t33, t23[:, :, 0:14],
                                            t23[:, :, 12:26], op=ALU.max)
                    t4 = sbp.tile([P, G * 8], BF16, tag="t4", name="t4",
                                  bufs=1)
                    t43 = t4.rearrange("p (g c) -> p g c", g=G)
                    nc.vector.tensor_tensor(t43, t33[:, :, 0:8],
                                            t33[:, :, 6:14], op=ALU.max)
                    t5 = sbp.tile([P, G * 4], BF16, tag="t5", name="t5",
                                  bufs=1)
                    t53 = t5.rearrange("p (g c) -> p g c", g=G)
                    nc.vector.tensor_tensor(t53, t43[:, :, 0:4],
                                            t43[:, :, 4:8], op=ALU.max)
                    t6 = sbp.tile([P, G * 2], BF16, tag="t6", name="t6",
                                  bufs=1)
                    t63 = t6.rearrange("p (g c) -> p g c", g=G)
                    nc.vector.tensor_tensor(t63, t53[:, :, 0:2],
                                            t53[:, :, 2:4], op=ALU.max)
                    nc.vector.tensor_tensor(emax2[:, h * G:(h + 1) * G],
                                            t63[:, :, 0:1].opt(),
                                            t63[:, :, 1:2].opt(), op=ALU.max)

                    # acc = E[label] >= emax (label class is column 0); early
                    # so E is freed before the next exp needs its buffer.
                    pack = sbp.tile([P, 2 * G], BF16, tag="pack", name="pack",
                                    bufs=4)
                    pack_h.append(pack)
                    pack4 = pack.rearrange("p (r g) -> p r g", r=4)
                    nc.vector.tensor_tensor(
                        pack4[:, 1:4:2, :],
                        E3[:, :, 0:1].opt().rearrange("p (u g) -> p u g", u=2),
                        emax2[:, h * G:(h + 1) * G].rearrange(
                            "p (u g) -> p u g", u=2), op=ALU.is_ge)

                # ---- pair end: reciprocal in transposed space, PE-transpose
                #      back, conf, masks, histogram matmuls
                for h in range(2):
                    RT = sbp.tile([P, P], F32, tag="RT", name="RT", bufs=2)
                    nc.vector.reciprocal_approx_fast(RT, ST_h[h])
                    RbT = sbp.tile([P, P], BF16, tag="RbT", name="RbT",
                                   bufs=2)
                    nc.vector.tensor_copy(RbT, RT)
                    stgR = psp.tile([P, 512], BF16, tag="stg", name="stgR",
                                    bufs=2)
                    nc.tensor.transpose(stgR[:, 0:P], RbT, ident_t)
                    Rbt = sbp.tile([P, P], BF16, tag="Rbt", name="Rbt",
                                   bufs=2)
                    nc.scalar.copy(Rbt, stgR[:, 0:P])
                    Rbt_h.append(Rbt)
                for h in range(2):
                    nc.vector.tensor_tensor(conf2[:, h * G:(h + 1) * G],
                                            emax2[:, h * G:(h + 1) * G],
                                            Rbt_h[h], op=ALU.mult)

                # cumulative bin masks: 15 tensor_scalar is_gt (4x mode)
                mask2 = sbp.tile([P, NBINS * 2 * G], BF16, tag="mask2",
                                 name="mask2")
                for b in range(NBINS):
                    nc.vector.tensor_scalar(
                        mask2[:, b * 2 * G:(b + 1) * 2 * G], conf2,
                        thr_imm[b], None, op0=ALU.is_gt)
                m3 = mask2.rearrange("p (b g) -> p b g", b=NBINS)

                for h in range(2):
                    t = 2 * tp + h
                    pack = pack_h[h]
                    pack4 = pack.rearrange("p (r g) -> p r g", r=4)
                    nc.vector.tensor_copy(
                        pack4[:, 0:3:2, :],
                        conf2[:, h * G:(h + 1) * G].rearrange(
                            "p (u g) -> p u g", u=2))

                    # histogram matmuls: 4 slot-quarters into 4 PSUM banks
                    for q in range(4):
                        u = q // 2
                        lhsT = pack[:, 128 * u:128 * u + 128]
                        rhs = m3[:, :, h * G + 32 * q:h * G + 32 * q + 32]
                        nc.tensor.matmul(hists[q], lhsT=lhsT, rhs=rhs,
                                         start=(t == 0), stop=(t == T - 1))

            # ---- finalize: dump histograms; host does the tiny reduction
            for q in range(4):
                hsb = sbp.tile([P, 32 * NBINS], F32, tag="hsb", name="hsb",
                               bufs=4)
                nc.scalar.copy(hsb, hists[q])
                nc.sync.dma_start(out_d.ap()[q], hsb)

    nc.compile()
    return nc


# ------------------------------------------------------------------- runner

_CACHE = {}


def _prepare(logits, labels, temperature, n_cores=N_CORES):
    labels = np.asarray(labels)
    key = hashlib.sha1(labels.tobytes()).hexdigest()
    if key in _CACHE:
        nc, slot_labels, core_idx, T = _CACHE[key]
    else:
        slot_labels, core_idx, T = build_plan(labels, n_cores)
        nc = build_program(T, n_cores)
        _CACHE[key] = (nc, slot_labels, core_idx, T)

    logits = np.asarray(logits, dtype=np.float32)
    tempr = np.broadcast_to(
        np.asarray(temperature, np.float32).ravel()[0:1], (P, 1)).copy()
    ident = np.eye(P, dtype=BFNP)
    # gmat block j, row p: one-hot of the slot that u3-column 128j+p feeds
    gmat = np.zeros((P, 14 * P), dtype=BFNP)
    cols = np.arange(14 * P)
    gmat[cols % P, (cols // P) * P + cols // 14] = BFNP(1.0)
    in_maps = []
    for c in range(n_cores):
        in_maps.append({
            "tempr": tempr,
            "ident": ident,
            "gmat": gmat,
            "logits": build_core_slab(logits, core_idx[c], slot_labels),
        })
    return nc, in_maps


def finalize_host(hists, n_total=N_TOTAL):
    """hists: list of per-core [4, P, 32*NBINS] f32. Returns ECE f32 [1]."""
    j = np.arange(32)
    sc_cum = np.zeros(NBINS, np.float64)
    sa_cum = np.zeros(NBINS, np.float64)
    for h in hists:
        h5 = np.asarray(h, np.float64).reshape(4, P, NBINS, 32)
        for q in range(4):
            r0 = 32 * (q % 2)
            sc_cum += h5[q, r0 + j, :, j].sum(axis=0)
            sa_cum += h5[q, 64 + r0 + j, :, j].sum(axis=0)
    sc = sc_cum - np.concatenate([sc_cum[1:], [0.0]])
    sa = sa_cum - np.concatenate([sa_cum[1:], [0.0]])
    ece = np.abs(sc - sa).sum() / float(n_total)
    return np.asarray([ece], dtype=np.float32)


def _ensure_ntff_hook():
    """This container's antenv lacks axon_hooks; synthesize it and register
    the ctypes NTFF hook so trace=True works under axon."""
    try:
        import antenv.axon_hooks  # noqa: F401
        return
    except ImportError:
        pass
    import types

    import antenv

    mod = types.ModuleType("antenv.axon_hooks")
    _hook = [None]
    mod.set_axon_ntff_profile_hook = lambda h: _hook.__setitem__(0, h)
    mod.get_axon_ntff_profile_hook = lambda: _hook[0]
    sys.modules["antenv.axon_hooks"] = mod
    antenv.axon_hooks = mod
    try:
        from trn_agent_boot.trn_boot import _ntff_profile_via_ctypes
        mod.set_axon_ntff_profile_hook(
            _ntff_profile_via_ctypes("/opt/axon/libaxon_pjrt.so"))
    except Exception:
        pass


def run(logits, labels, temperature, n_total=None, trace=False,
        n_cores=N_CORES):
    if trace:
        _ensure_ntff_hook()
    if n_total is None:
        n_total = int(np.asarray(labels).shape[0])
    nc, in_maps = _prepare(logits, labels, temperature, n_cores)
    res = bass_utils.run_bass_kernel_spmd(
        nc, in_maps, core_ids=list(range(n_cores)), trace=trace)
    out = finalize_host([r["out"] for r in res.results], n_total)
    return out, res


def kernel(logits, labels, temperature):
    out, _ = run(logits, labels, temperature)
    return out


# revision 18
# speedup vs baseline: 1.3411x; 1.3411x over previous
"""ECE loss kernel for Trainium2, data-parallel over 8 NeuronCores.

Host side shards + permutes samples (binning is permutation invariant) into
128-sample single-label "slots", swaps each slot's label class into column 0
(max/sum are class-permutation invariant, so accuracy becomes one column
compare), and casts logits to bf16 (ECE rel-err from bf16 ~3e-3, tol 2e-2).

Device per tile (128 partitions x 128 slots x 104 padded classes, bf16):
  ScalarE: one big exp(logits/T) instruction (13312 elems/partition).
  DVE:     pairwise tensor_tensor max/add trees (bf16 2x mode) down to
           per-sample max-exp and softmax denominators, fast reciprocal,
           conf/acc, and tile-pair-batched tensor_scalar threshold compares.
  PE:      four matmuls per tile accumulate per-(slot, bin) sums of
           (conf, acc) into four PSUM banks across all tiles.
Each core DMAs its [4,128,480] histogram out; the host extracts the diagonal
slot blocks, reduces 8 cores' 15-bin stats, and finishes ECE in float64.
"""

import hashlib
import sys

import numpy as np

sys.path.insert(0, "/opt/trn_rl_repo")

import ml_dtypes  # noqa: E402

from concourse import bacc, bass, mybir, tile  # noqa: E402
from concourse import bass_utils  # noqa: E402

P = 128          # partitions
G = 128          # slots (groups) per tile
TILE = P * G     # samples per tile (16384)
C = 100          # classes
CP = 104         # padded class stride (4B-aligned bf16 rows, tree-friendly)
NBINS = 15
N_CORES = 8
BIG = 80.0       # pad logit; exp(-80) ~ 1.8e-35 is harmless in f32/bf16
N_TOTAL = 2_000_000

F32 = mybir.dt.float32
BF16 = mybir.dt.bfloat16
BFNP = ml_dtypes.bfloat16
AX = mybir.AxisListType
ALU = mybir.AluOpType
ACTF = mybir.ActivationFunctionType


# ---------------------------------------------------------------- host layout

def build_plan(labels: np.ndarray, n_cores: int = N_CORES):
    """Deal samples round-robin per label so every core has the same number
    of 128-sample slots per label.  Returns (slot_labels, per-core sample
    index arrays with -1 for pad rows, tile count)."""
    labels = np.asarray(labels).astype(np.int64).ravel()
    order = np.argsort(labels, kind="stable")
    sorted_labels = labels[order]
    starts = np.searchsorted(sorted_labels, np.arange(C))
    ends = np.searchsorted(sorted_labels, np.arange(C), side="right")

    slot_labels = []
    core_chunks = [[] for _ in range(n_cores)]
    for k in range(C):
        idx_k = order[starts[k]:ends[k]]
        per_core = [idx_k[c::n_cores] for c in range(n_cores)]
        max_cnt = max(len(x) for x in per_core)
        slots_k = -(-max_cnt // P) if max_cnt > 0 else 0
        if slots_k == 0:
            continue
        padded = slots_k * P
        for c in range(n_cores):
            buf = np.full(padded, -1, dtype=np.int64)
            buf[: len(per_core[c])] = per_core[c]
            core_chunks[c].append(buf)
        slot_labels.extend([k] * slots_k)

    n_slots = len(slot_labels)
    pad_slots = (-n_slots) % (2 * G)
    if pad_slots:
        for c in range(n_cores):
            core_chunks[c].append(np.full(pad_slots * P, -1, dtype=np.int64))
        slot_labels.extend([0] * pad_slots)
        n_slots += pad_slots

    slot_labels = np.asarray(slot_labels, dtype=np.int64)
    core_idx = [np.concatenate(ch) for ch in core_chunks]
    T = n_slots // G
    return slot_labels, core_idx, T


def build_core_slab(logits: np.ndarray, idx: np.ndarray,
                    slot_labels: np.ndarray) -> np.ndarray:
    """One core's [T*TILE, CP] bf16 slab in device DMA order, with each
    slot's label class swapped into column 0."""
    S = len(slot_labels)
    arr = np.full((S * P, CP), -BIG, dtype=BFNP)
    arr[:, :C] = logits[np.maximum(idx, 0)].astype(BFNP)
    # swap label class into column 0 (rows of slot s all have the same label)
    ks = np.repeat(slot_labels, P)
    rows = np.arange(S * P)
    col0 = arr[rows, 0].copy()
    arr[rows, 0] = arr[rows, ks]
    arr[rows, ks] = col0
    pad_pos = np.nonzero(idx < 0)[0]
    if len(pad_pos):
        arr[pad_pos, :C] = BFNP(-BIG)
        arr[pad_pos, 0] = BFNP(BIG)
    # [S, P, CP] slot-major -> [T, P, G, CP] DMA order
    arr = arr.reshape(S // G, G, P, CP).transpose(0, 2, 1, 3)
    return np.ascontiguousarray(arr).reshape(-1, CP)


# ------------------------------------------------------------- device program

def build_program(T: int, n_cores: int = N_CORES):
    nc = bacc.Bacc("TRN2", target_bir_lowering=False, debug=False,
                   num_devices=n_cores)

    logits_d = nc.dram_tensor("logits", [T * TILE, CP], BF16,
                              kind="ExternalInput")
    tempr_d = nc.dram_tensor("tempr", [P, 1], F32, kind="ExternalInput")
    out_d = nc.dram_tensor("out", [4, P, 32 * NBINS], F32,
                           kind="ExternalOutput")

    thr_imm = [float(np.float32(BFNP(b / NBINS))) for b in range(NBINS)]
    assert T % 2 == 0

    with tile.TileContext(nc) as tc:
        with (
            tc.tile_pool(name="const", bufs=1) as const,
            tc.tile_pool(name="rawp", bufs=2) as rawp,
            tc.tile_pool(name="sb", bufs=2) as sbp,
            tc.tile_pool(name="ps", bufs=1, space="PSUM") as psp,
        ):
            tempr_t = const.tile([P, 1], F32)
            nc.sync.dma_start(tempr_t, tempr_d.ap())
            invT = const.tile([P, 1], F32)
            nc.vector.reciprocal(invT, tempr_t)
            warm = const.tile([P, 1], F32)
            nc.scalar.activation(warm, tempr_t, ACTF.Exp)

            hists = [psp.tile([P, 32 * NBINS], F32, name=f"hist{q}")
                     for q in range(4)]

            logits_ap = logits_d.ap()
            for tp in range(T // 2):
                emax2 = sbp.tile([P, 2 * G], BF16, tag="emax2", name="emax2",
                                 bufs=1)
                S2 = sbp.tile([P, 2 * G], F32, tag="S2", name="S2", bufs=1)
                pack_h = []
                for h in range(2):
                    t = 2 * tp + h
                    raw = rawp.tile([P, G * CP], BF16, tag="raw", name="raw")
                    E = sbp.tile([P, G * CP], BF16, tag="E", name="E")
                    E3 = E.rearrange("p (g c) -> p g c", g=G)
                    t1 = sbp.tile([P, G * 52], BF16, tag="t1", name="t1",
                                  bufs=1)
                    t13 = t1.rearrange("p (g c) -> p g c", g=G)
                    u1 = sbp.tile([P, G * 52], BF16, tag="u1", name="u1",
                                  bufs=1)
                    u13 = u1.rearrange("p (g c) -> p g c", g=G)

                    # Tile 0 is sub-chunked so DVE starts ~25us earlier.
                    src = logits_ap[t * TILE:(t + 1) * TILE,
                                    :].rearrange("(p s) c -> p (s c)", p=P)
                    nsub = {0: 8, 1: 4, 2: 2}.get(t, 1)
                    gs = G // nsub
                    for c in range(nsub):
                        fsl = slice(c * gs * CP, (c + 1) * gs * CP)
                        gsl = slice(c * gs, (c + 1) * gs)
                        nc.sync.dma_start(raw[:, fsl], src[:, fsl])
                        nc.scalar.activation(E[:, fsl], raw[:, fsl], ACTF.Exp,
                                             scale=invT)
                        # tree L1 (bf16 2x TT): max and sum of class pairs
                        nc.vector.tensor_tensor(t13[:, gsl, :],
                                                E3[:, gsl, 0:52],
                                                E3[:, gsl, 52:104], op=ALU.max)
                        nc.vector.tensor_tensor(u13[:, gsl, :],
                                                E3[:, gsl, 0:52],
                                                E3[:, gsl, 52:104], op=ALU.add)

                    # ---- max tree L2..L7 (overlapped splits keep alignment)
                    t2 = sbp.tile([P, G * 26], BF16, tag="t2", name="t2",
                                  bufs=1)
                    t23 = t2.rearrange("p (g c) -> p g c", g=G)
                    nc.vector.tensor_tensor(t23, t13[:, :, 0:26],
                                            t13[:, :, 26:52], op=ALU.max)
                    t3 = sbp.tile([P, G * 14], BF16, tag="t3", name="t3",
                                  bufs=1)
                    t33 = t3.rearrange("p (g c) -> p g c", g=G)
                    nc.vector.tensor_tensor(t33, t23[:, :, 0:14],
                                            t23[:, :, 12:26], op=ALU.max)
                    t4 = sbp.tile([P, G * 8], BF16, tag="t4", name="t4",
                                  bufs=1)
                    t43 = t4.rearrange("p (g c) -> p g c", g=G)
                    nc.vector.tensor_tensor(t43, t33[:, :, 0:8],
                                            t33[:, :, 6:14], op=ALU.max)
                    t5 = sbp.tile([P, G * 4], BF16, tag="t5", name="t5",
                                  bufs=1)
                    t53 = t5.rearrange("p (g c) -> p g c", g=G)
                    nc.vector.tensor_tensor(t53, t43[:, :, 0:4],
                                            t43[:, :, 4:8], op=ALU.max)
                    t6 = sbp.tile([P, G * 2], BF16, tag="t6", name="t6",
                                  bufs=1)
                    t63 = t6.rearrange("p (g c) -> p g c", g=G)
                    nc.vector.tensor_tensor(t63, t53[:, :, 0:2],
                                            t53[:, :, 2:4], op=ALU.max)
                    nc.vector.tensor_tensor(emax2[:, h * G:(h + 1) * G],
                                            t63[:, :, 0:1].opt(),
                                            t63[:, :, 1:2].opt(), op=ALU.max)

                    # acc = E[label] >= emax (label class is column 0); doing
                    # this here frees E before the sum tree, keeping ScalarE's
                    # next exp pipelined.  pack layout [conf0|acc0|conf1|acc1]
                    # so each matmul's lhsT is one contiguous 128-col slice.
                    pack = sbp.tile([P, 2 * G], BF16, tag="pack", name="pack",
                                    bufs=4)
                    pack_h.append(pack)
                    pack4 = pack.rearrange("p (r g) -> p r g", r=4)
                    nc.vector.tensor_tensor(
                        pack4[:, 1:4:2, :],
                        E3[:, :, 0:1].opt().rearrange("p (u g) -> p u g", u=2),
                        emax2[:, h * G:(h + 1) * G].rearrange(
                            "p (u g) -> p u g", u=2), op=ALU.is_ge)

                    # ---- sum tree L2..L7 (no overlap; odd tails pass)
                    u2 = sbp.tile([P, G * 26], BF16, tag="u2", name="u2",
                                  bufs=1)
                    u23 = u2.rearrange("p (g c) -> p g c", g=G)
                    nc.vector.tensor_tensor(u23, u13[:, :, 0:26],
                                            u13[:, :, 26:52], op=ALU.add)
                    u3 = sbp.tile([P, G * 14], BF16, tag="u3", name="u3",
                                  bufs=1)
                    u33 = u3.rearrange("p (g c) -> p g c", g=G)
                    nc.vector.tensor_tensor(u33[:, :, 0:12], u23[:, :, 0:12],
                                            u23[:, :, 14:26], op=ALU.add)
                    nc.vector.tensor_copy(u33[:, :, 12:14], u23[:, :, 12:14])
                    u4 = sbp.tile([P, G * 8], BF16, tag="u4", name="u4",
                                  bufs=1)
                    u43 = u4.rearrange("p (g c) -> p g c", g=G)
                    nc.vector.tensor_tensor(u43[:, :, 0:6], u33[:, :, 0:6],
                                            u33[:, :, 8:14], op=ALU.add)
                    nc.vector.tensor_copy(u43[:, :, 6:8], u33[:, :, 6:8])
                    u5 = sbp.tile([P, G * 4], BF16, tag="u5", name="u5",
                                  bufs=1)
                    u53 = u5.rearrange("p (g c) -> p g c", g=G)
                    nc.vector.tensor_tensor(u53, u43[:, :, 0:4],
                                            u43[:, :, 4:8], op=ALU.add)
                    u6 = sbp.tile([P, G * 2], BF16, tag="u6", name="u6",
                                  bufs=1)
                    u63 = u6.rearrange("p (g c) -> p g c", g=G)
                    nc.vector.tensor_tensor(u63, u53[:, :, 0:2],
                                            u53[:, :, 2:4], op=ALU.add)
                    nc.vector.tensor_tensor(S2[:, h * G:(h + 1) * G],
                                            u63[:, :, 0:1].opt(),
                                            u63[:, :, 1:2].opt(), op=ALU.add)

                # ---- per-sample math, batched across the tile pair
                R2 = sbp.tile([P, 2 * G], F32, tag="R2", name="R2", bufs=1)
                nc.vector.reciprocal_approx_fast(R2, S2)
                Rb2 = sbp.tile([P, 2 * G], BF16, tag="Rb2", name="Rb2",
                               bufs=1)
                nc.vector.tensor_copy(Rb2, R2)
                conf2 = sbp.tile([P, 2 * G], BF16, tag="conf2", name="conf2",
                                 bufs=1)
                nc.vector.tensor_tensor(conf2, emax2, Rb2, op=ALU.mult)

                # cumulative bin masks: 15 tensor_scalar is_gt (4x mode)
                mask2 = sbp.tile([P, NBINS * 2 * G], BF16, tag="mask2",
                                 name="mask2")
                for b in range(NBINS):
                    nc.vector.tensor_scalar(
                        mask2[:, b * 2 * G:(b + 1) * 2 * G], conf2,
                        thr_imm[b], None, op0=ALU.is_gt)
                m3 = mask2.rearrange("p (b g) -> p b g", b=NBINS)

                for h in range(2):
                    t = 2 * tp + h
                    pack = pack_h[h]
                    pack4 = pack.rearrange("p (r g) -> p r g", r=4)
                    nc.vector.tensor_copy(
                        pack4[:, 0:3:2, :],
                        conf2[:, h * G:(h + 1) * G].rearrange(
                            "p (u g) -> p u g", u=2))

                    # histogram matmuls: 4 slot-quarters into 4 PSUM banks
                    for q in range(4):
                        u = q // 2
                        lhsT = pack[:, 128 * u:128 * u + 128]
                        rhs = m3[:, :, h * G + 32 * q:h * G + 32 * q + 32]
                        nc.tensor.matmul(hists[q], lhsT=lhsT, rhs=rhs,
                                         start=(t == 0), stop=(t == T - 1))

            # ---- finalize: dump histograms; host does the tiny reduction
            for q in range(4):
                hsb = sbp.tile([P, 32 * NBINS], F32, tag="hsb", name="hsb",
                               bufs=4)
                nc.scalar.copy(hsb, hists[q])
                nc.sync.dma_start(out_d.ap()[q], hsb)

    nc.compile()
    return nc


# ------------------------------------------------------------------- runner

_CACHE = {}


def _prepare(logits, labels, temperature, n_cores=N_CORES):
    labels = np.asarray(labels)
    key = hashlib.sha1(labels.tobytes()).hexdigest()
    if key in _CACHE:
        nc, slot_labels, core_idx, T = _CACHE[key]
    else:
        slot_labels, core_idx, T = build_plan(labels, n_cores)
        nc = build_program(T, n_cores)
        _CACHE[key] = (nc, slot_labels, core_idx, T)

    logits = np.asarray(logits, dtype=np.float32)
    tempr = np.broadcast_to(
        np.asarray(temperature, np.float32).ravel()[0:1], (P, 1)).copy()
    in_maps = []
    for c in range(n_cores):
        in_maps.append({
            "tempr": tempr,
            "logits": build_core_slab(logits, core_idx[c], slot_labels),
        })
    return nc, in_maps


def finalize_host(hists, n_total=N_TOTAL):
    """hists: list of per-core [4, P, 32*NBINS] f32. Returns ECE f32 [1]."""
    j = np.arange(32)
    sc_cum = np.zeros(NBINS, np.float64)
    sa_cum = np.zeros(NBINS, np.float64)
    for h in hists:
        h5 = np.asarray(h, np.float64).reshape(4, P, NBINS, 32)
        for q in range(4):
            r0 = 32 * (q % 2)
            sc_cum += h5[q, r0 + j, :, j].sum(axis=0)
            sa_cum += h5[q, 64 + r0 + j, :, j].sum(axis=0)
    sc = sc_cum - np.concatenate([sc_cum[1:], [0.0]])
    sa = sa_cum - np.concatenate([sa_cum[1:], [0.0]])
    ece = np.abs(sc - sa).sum() / float(n_total)
    return np.asarray([ece], dtype=np.float32)


def _ensure_ntff_hook():
    """This container's antenv lacks axon_hooks; synthesize it and register
    the ctypes NTFF hook so trace=True works under axon."""
    try:
        import antenv.axon_hooks  # noqa: F401
        return
    except ImportError:
        pass
    import types

    import antenv

    mod = types.ModuleType("antenv.axon_hooks")
    _hook = [None]
    mod.set_axon_ntff_profile_hook = lambda h: _hook.__setitem__(0, h)
    mod.get_axon_ntff_profile_hook = lambda: _hook[0]
    sys.modules["antenv.axon_hooks"] = mod
    antenv.axon_hooks = mod
    try:
        from trn_agent_boot.trn_boot import _ntff_profile_via_ctypes
        mod.set_axon_ntff_profile_hook(
            _ntff_profile_via_ctypes("/opt/axon/libaxon_pjrt.so"))
    except Exception:
        pass


def run(logits, labels, temperature, n_total=None, trace=False,
        n_cores=N_CORES):
    if trace:
        _ensure_ntff_hook()
    if n_total is None:
        n_total = int(np.asarray(labels).shape[0])
    nc, in_maps = _prepare(logits, labels, temperature, n_cores)
    res = bass_utils.run_bass_kernel_spmd(
        nc, in_maps, core_ids=list(range(n_cores)), trace=trace)
    out = finalize_host([r["out"] for r in res.results], n_total)
    return out, res


def kernel(logits, labels, temperature):
    out, _ = run(logits, labels, temperature)
    return out
